# revision 24
# baseline (speedup 1.0000x reference)
"""Multi-head attention (B=2, L=2048, D=1024, H=16, DK=64) on 8 TRN2 NeuronCores.

Sharding: core c handles batch b = c//4 and head-group g = c%4 (4 heads,
256 model dims). Per-core compute (no collectives):
  QT/KT  [256, 2048] projections in [dk, seq] layout (rhs = x^T, lhsT = w^T)
  V      [2048, 256] with a fused ones-column per head (gives softmax Z free)
  S^T    = K_h @ Q_h^T per head in [keys, queries] layout (row-packed head
           pairs on the PE array, K=64 each)
  P      = exp(S^T / 8)     one wide ACT op per (key-tile, head-pair)
  ctx^T  = V'_h^T @ P       -> rows 0:64 ctx, row 64 = Z
  ctx   /= Z                (batched reciprocal + gpsimd partition broadcast)
  out_g  = ctx_g @ w_o[:, g]^T   -> per-core PARTIAL output [2048, 1024]
Host sums the 4 head-group partials per batch and stacks the 2 batches.
"""

import numpy as np

D = 1024
L = 2048
DK = 64
GH = 4           # heads per core
GD = GH * DK     # model dims per core (256)
NCORES = 8

MM_DTYPE = "bfloat16"   # matmul operand dtype: bfloat16 | float32r | float32


def _build(mm_dtype: str = MM_DTYPE):
    import concourse.bacc as bacc
    import concourse.mybir as mybir
    import concourse.tile as tile

    f32 = mybir.dt.float32
    mmdt = getattr(mybir.dt, mm_dtype)
    Exp = mybir.ActivationFunctionType.Exp

    # DRAM input dtype: bf16 ships converted data; f32r ships f32 bits
    # (pre-rounded on host) and bitcasts the DMA source AP.
    ddt = mmdt if mm_dtype == "bfloat16" else f32

    nc = bacc.Bacc("TRN2", target_bir_lowering=False, debug=False,
                   num_devices=NCORES)
    xT = nc.dram_tensor("xT", [D, L], ddt, kind="ExternalInput").ap()
    wqT = nc.dram_tensor("wqT", [D, GD], ddt, kind="ExternalInput").ap()
    wkT = nc.dram_tensor("wkT", [D, GD], ddt, kind="ExternalInput").ap()
    wvT = nc.dram_tensor("wvT", [D, GD], ddt, kind="ExternalInput").ap()
    woT = nc.dram_tensor("woT", [GD, D], ddt, kind="ExternalInput").ap()
    out = nc.dram_tensor("out", [L, D], f32, kind="ExternalOutput").ap()

    mdt = mmdt           # dtype for matmul-feeding SBUF tiles

    def dr(t):           # reinterpret a DRAM f32 AP as the matmul dtype
        return t.bitcast(mmdt) if mm_dtype == "float32r" else t

    ND = D // 128    # 8 d-tiles
    NL = L // 128    # 16 key tiles
    NQ = L // 512    # 4 query chunks

    with tile.TileContext(nc) as tc:
        with (
            tc.tile_pool(name="xp", bufs=1) as xp,
            tc.tile_pool(name="wp", bufs=1) as wp,
            tc.tile_pool(name="qk", bufs=1) as qk,
            tc.tile_pool(name="vp", bufs=1) as vp,
            tc.tile_pool(name="cx", bufs=1) as cx,
            tc.tile_pool(name="pp", bufs=14) as pp,
            tc.tile_pool(name="rp", bufs=8) as rp,
            tc.tile_pool(name="op", bufs=4) as op_,
            tc.tile_pool(name="ps", bufs=2, space="PSUM") as ps,
            tc.tile_pool(name="pc", bufs=2, space="PSUM") as pc,
            tc.tile_pool(name="po", bufs=2, space="PSUM") as pop,
        ):
            # ---- DMA inputs -------------------------------------------------
            wqts = wp.tile([128, ND, GD], mdt, tag="wq", name="wq")
            wkts = wp.tile([128, ND, GD], mdt, tag="wk", name="wk")
            wvts = wp.tile([128, ND, GD], mdt, tag="wv", name="wv")
            wots = [wp.tile([128, D], mdt, tag=f"wo{i}", name=f"wo{i}")
                    for i in range(2)]
            for w_dram, w_sb in ((wqT, wqts), (wkT, wkts)):
                r = w_dram.rearrange("(d p) c -> p d c", p=128)
                for i in range(2):
                    nc.gpsimd.dma_start(w_sb[:, 4 * i:4 * i + 4, :],
                                        dr(r[:, 4 * i:4 * i + 4, :]))
            xts = [xp.tile([128, L], mdt, tag=f"x{d}", name=f"x{d}")
                   for d in range(ND)]
            for d in range(ND):
                nc.sync.dma_start(xts[d][:, 0:256],
                                  dr(xT[d * 128:(d + 1) * 128, 0:256]))
            for d in range(ND):
                nc.sync.dma_start(xts[d][:, 256:512],
                                  dr(xT[d * 128:(d + 1) * 128, 256:512]))
            for qtr in range(1, 4):
                sl = slice(qtr * 512, (qtr + 1) * 512)
                eng = nc.scalar if qtr % 2 == 1 else nc.sync
                for d in range(ND):
                    eng.dma_start(xts[d][:, sl],
                                  dr(xT[d * 128:(d + 1) * 128, sl]))
            r = wvT.rearrange("(d p) c -> p d c", p=128)
            for i in range(2):
                nc.gpsimd.dma_start(wvts[:, 4 * i:4 * i + 4, :],
                                    dr(r[:, 4 * i:4 * i + 4, :]))
            for i in range(2):
                nc.gpsimd.dma_start(wots[i][:], dr(woT[i * 128:(i + 1) * 128, :]))

            # ---- projection helpers (emitted just-in-time) ----------------
            vph = [vp.tile([128, NL, DK + 1], mdt, tag=f"v{h}", name=f"v{h}")
                   for h in range(GH)]
            onesc = wp.tile([128, NL, 1], f32, tag="ones", name="ones")
            nc.vector.memset(onesc[:], 1.0)
            for h in range(GH):
                nc.vector.tensor_copy(vph[h][:, :, DK:DK + 1], onesc[:])

            qth = [qk.tile([128, L], mdt, tag=f"q{hp}", name=f"q{hp}")
                   for hp in range(2)]
            kth = [qk.tile([128, L], mdt, tag=f"k{hp}", name=f"k{hp}")
                   for hp in range(2)]
            ctxt = [cx.tile([128, L], mdt, tag=f"c{hp}", name=f"c{hp}")
                    for hp in range(2)]

            def proj_qk_chunk(hp, w_sb, dstl, qc):
                acc = pop.tile([128, 512], f32, tag="o", name="o")
                for d in range(ND):
                    nc.tensor.matmul(
                        acc[:], w_sb[:, d, hp * 128:(hp + 1) * 128],
                        xts[d][:, qc * 512:(qc + 1) * 512],
                        start=(d == 0), stop=(d == ND - 1))
                nc.vector.tensor_copy(
                    dstl[hp][:, qc * 512:(qc + 1) * 512], acc[:])

            def v_proj_tile(lt):
                acc = pop.tile([128, 512], f32, tag="o", name="o")
                for d in range(ND):
                    nc.tensor.matmul(
                        acc[:, 0:GD],
                        xts[d][:, lt * 128:(lt + 1) * 128],
                        wvts[:, d, :],
                        start=(d == 0), stop=(d == ND - 1))
                for h in range(GH):
                    nc.vector.tensor_copy(
                        vph[h][:, lt, 0:DK],
                        acc[:, h * DK:(h + 1) * DK])

            def out_proj(qc, after=None):
                from concourse.tile import add_dep_helper
                unit = 0
                for qt in range(4):
                    rows = slice(qc * 512 + qt * 128,
                                 qc * 512 + (qt + 1) * 128)
                    ot = op_.tile([128, 1024], f32, tag="ot", name="ot")
                    for ec in range(2):
                        esl = slice(ec * 512, (ec + 1) * 512)
                        po = pop.tile([128, 512], f32, tag="o", name="o")
                        for hp in range(2):
                            mm = nc.tensor.matmul(
                                po[:], ctxt[hp][:, rows],
                                wots[hp][:, esl],
                                start=(hp == 0), stop=(hp == 1))
                            if hp == 0 and after is not None:
                                pin = after[min(2 * unit + 1, len(after) - 1)]
                                add_dep_helper(mm.ins, pin.ins, sync=False,
                                               reason="pipeline out_proj")
                        unit += 1
                        nc.vector.tensor_copy(ot[:, esl], po[:])
                    nc.sync.dma_start(out[rows, :], ot[:])

            def attn(hp, qc, interleave=None):
                """Attention for head-pair hp, query chunk qc.

                interleave: dict lt -> callable emitting extra PE work after
                the lt's score matmuls (keeps PE busy while ACT churns).
                Ends with the Z-division for this head pair. Returns the
                per-lt ctx matmuls for ordering pins.
                """
                qsl = slice(qc * 512, (qc + 1) * 512)
                cps = [pc.tile([DK + 1, 512], f32, tag="c", name=f"c{i}")
                       for i in range(2)]
                ctx_mms = []
                for lt in range(NL):
                    lsl = slice(lt * 128, (lt + 1) * 128)
                    sp = ps.tile([128, 1024], f32, tag="s", name="s")
                    for i in range(2):
                        row = slice(i * 64, (i + 1) * 64)
                        nc.tensor.matmul(
                            sp[:, i * 512:(i + 1) * 512],
                            kth[hp][row, lsl], qth[hp][row, qsl],
                            start=True, stop=True,
                            tile_position=(i * 64, 0))
                    if interleave is not None and lt in interleave:
                        interleave[lt]()
                    p = pp.tile([128, 1024], mdt, tag="p", name="p")
                    nc.scalar.activation(p[:], sp[:], Exp, scale=0.125)
                    for i in range(2):
                        mm = nc.tensor.matmul(
                            cps[i][:], vph[2 * hp + i][:, lt, :],
                            p[:, i * 512:(i + 1) * 512],
                            start=(lt == 0), stop=(lt == NL - 1))
                        if i == 1:
                            ctx_mms.append(mm)
                for i in range(2):
                    cu = pp.tile([64, 512], mdt, tag=f"cu{2*hp+i}",
                                 name=f"cu{2*hp+i}", bufs=2)
                    nc.vector.tensor_copy(cu[:], cps[i][0:DK, :])
                    zi = rp.tile([1, 512], f32, tag="zi", name="zi")
                    nc.vector.tensor_copy(zi[:], cps[i][DK:DK + 1, :])
                    rz = rp.tile([1, 512], f32, tag="rz", name="rz")
                    nc.vector.reciprocal_approx_fast(rz[:], zi[:])
                    rzb = rp.tile([64, 512], f32, tag="rzb", name="rzb")
                    nc.gpsimd.partition_broadcast(rzb[:], rz[:])
                    nc.vector.tensor_mul(
                        ctxt[hp][i * 64:(i + 1) * 64, qsl],
                        cu[:], rzb[:])
                return ctx_mms

            # ---- schedule ---------------------------------------------------
            for qc in range(NQ):
                proj_qk_chunk(0, wkts, kth, qc)
            proj_qk_chunk(0, wqts, qth, 0)

            qk_units = [
                lambda: proj_qk_chunk(0, wqts, qth, 1),
                lambda: proj_qk_chunk(0, wqts, qth, 2),
                lambda: proj_qk_chunk(0, wqts, qth, 3),
                lambda: proj_qk_chunk(1, wkts, kth, 0),
                lambda: proj_qk_chunk(1, wkts, kth, 1),
                lambda: proj_qk_chunk(1, wkts, kth, 2),
                lambda: proj_qk_chunk(1, wkts, kth, 3),
                lambda: proj_qk_chunk(1, wqts, qth, 0),
            ]

            def il0_unit(lt):
                def f():
                    v_proj_tile(lt)
                    if lt % 2 == 1 and lt // 2 < len(qk_units):
                        qk_units[lt // 2]()
                return f
            attn(0, 0, interleave={lt: il0_unit(lt) for lt in range(NL)})
            il1 = {1: lambda: proj_qk_chunk(1, wqts, qth, 1),
                   5: lambda: proj_qk_chunk(1, wqts, qth, 2),
                   9: lambda: proj_qk_chunk(1, wqts, qth, 3)}
            attn(1, 0, interleave=il1)
            for qc in range(1, NQ):
                pins = attn(0, qc)
                out_proj(qc - 1, after=pins)
                attn(1, qc)
            out_proj(NQ - 1)
    nc.compile()
    return nc


_CACHED = {}


def _get_nc(mm_dtype: str = MM_DTYPE):
    if mm_dtype not in _CACHED:
        _CACHED[mm_dtype] = _build(mm_dtype)
    return _CACHED[mm_dtype]


def _round_fp32r(a):
    """Round-to-nearest-even fp32 -> fp32r (11 explicit mantissa bits)."""
    u = np.ascontiguousarray(a, np.float32).view(np.uint32).copy()
    u += 0x7FF + ((u >> 12) & 1)
    u &= 0xFFFFF000
    return u.view(np.float32)


def make_in_maps(x, w_qkv, w_o):
    if MM_DTYPE == "float32r":
        cvt = _round_fp32r
    elif MM_DTYPE == "bfloat16":
        import ml_dtypes
        cvt = lambda a: np.asarray(a, dtype=ml_dtypes.bfloat16)  # noqa: E731
    else:
        cvt = lambda a: a  # noqa: E731
    wq, wk, wv = (w_qkv[i * D:(i + 1) * D] for i in range(3))
    in_maps = []
    for c in range(NCORES):
        b, g = divmod(c, 4)
        gs = slice(g * GD, (g + 1) * GD)
        in_maps.append({
            "xT": cvt(np.ascontiguousarray(x[b].T)),
            "wqT": cvt(np.ascontiguousarray(wq[gs].T)),
            "wkT": cvt(np.ascontiguousarray(wk[gs].T)),
            "wvT": cvt(np.ascontiguousarray(wv[gs].T)),
            "woT": cvt(np.ascontiguousarray(w_o[:, gs].T)),
        })
    return in_maps


def assemble(results):
    out = np.empty((2, L, D), np.float32)
    for b in range(2):
        out[b] = sum(results[4 * b + g]["out"] for g in range(4))
    return out


def kernel(x, w_qkv, w_o):
    from concourse import bass_utils
    nc = _get_nc()
    in_maps = make_in_maps(np.asarray(x, np.float32),
                           np.asarray(w_qkv, np.float32),
                           np.asarray(w_o, np.float32))
    res = bass_utils.run_bass_kernel_spmd(
        nc, in_maps, core_ids=list(range(NCORES)))
    return assemble(res.results)
